# revision 10
# baseline (speedup 1.0000x reference)
"""Fused cross-entropy label-propagation kernel for Trainium2 (8 cores).

Computation (per batch b):
  sim   = ref_flat(b) @ tgt_flat(b)          # [12288, 4096]
  prob  = softmax(sim, axis=0)               # over ref pixels
  pred  = lab_flat(b) @ prob                 # [16, 4096]
  loss  = mean(-log(pred[label] + eps))

Sharding: batch b = core // 4, target-pixel columns split 4-way per batch
(softmax is over the ref axis, so column sharding needs no communication).

Device kernel (per core, T_LOC = 1024 columns):
  for each of 96 ref-row tiles (128 rows):
    sim_psum[128, 1024]  = refT_tile.T @ tgt          (2 accumulating matmuls, K=256)
    p[128, 1024]         = exp(sim_psum - 80)         (ScalarE, reads PSUM)
    pred_psum[17, 1024] += [lab|ones]_tile.T @ p      (accumulate over all 96 tiles)
  out = pred_psum   # rows 0..15 = unnormalized label sums, row 16 = softmax denom

The constant shift replaces the per-column max. The data contains near-collinear
ref/target pairs, so per-column sim maxima span roughly [57, 221] — that 164-wide
log-range just fits inside float32's ~175-unit representable exp window when
centered with shift ~138.5. The shift is a kernel INPUT: the host validates every
column of the result (denominator finite and positive) and, if any column falls
outside the window, reruns with the shift moved +-60 and patches those columns.
The shift cancels exactly in num/den.

Host finishes with num/den, log, gather, mean over 8192 elements (float64).
"""

import numpy as np

B, NREF, F, H, W, D = 2, 3, 256, 64, 64, 16
T = H * W                     # 4096 target pixels per batch
N = NREF * T                  # 12288 ref pixels per batch
NCORES = 8
T_LOC = B * T // NCORES       # 1024 columns per core
NT = N // 128                 # 96 ref-row tiles
NCHUNK = 4                    # ref DMA chunks per (r, fh) tile
CW = T // NCHUNK              # 1024 columns per ref chunk
SHIFT0 = 138.5                # subtracted from sim before exp (host-adjustable)
EPS = 1e-14

_CACHE = {}
LAST_RESULTS = None  # BassKernelResults of the most recent run (for profiling)


def _build_program():
    if "nc" in _CACHE:
        return _CACHE["nc"]

    import concourse.bacc as bacc
    import concourse.tile as tile
    import concourse.mybir as mybir

    f32 = mybir.dt.float32
    f32r = mybir.dt.float32r

    nc = bacc.Bacc("TRN2", target_bir_lowering=False, debug=False,
                   num_devices=NCORES)

    # Per-core inputs, pre-laid-out on host so every DMA is contiguous.
    ref_d = nc.dram_tensor("ref", [NREF, 2, NCHUNK, 128, CW], f32r,
                           kind="ExternalInput")
    tgt_d = nc.dram_tensor("tgt", [2, 128, T_LOC], f32r, kind="ExternalInput")
    lab_d = nc.dram_tensor("lab", [128, NT * (D + 1)], f32r,
                           kind="ExternalInput")
    shv_d = nc.dram_tensor("shv", [128, 1], f32, kind="ExternalInput")
    out_d = nc.dram_tensor("out", [D + 1, T_LOC], f32, kind="ExternalOutput")

    with tile.TileContext(nc) as tc:
        with (
            tc.tile_pool(name="refs", bufs=1) as refs,
            tc.tile_pool(name="small", bufs=1) as small,
            tc.tile_pool(name="ppool", bufs=3) as ppool,
            tc.tile_pool(name="simpool", bufs=3, space="PSUM") as simpool,
            tc.tile_pool(name="predpool", bufs=1, space="PSUM") as predpool,
        ):
            # Loads needed by the first iteration go first.
            tgt_sb = []
            for fh in range(2):
                t_ = small.tile([128, T_LOC], f32r, tag=f"tgt{fh}")
                nc.sync.dma_start(out=t_, in_=tgt_d[fh])
                tgt_sb.append(t_)
            lab_sb = small.tile([128, NT * (D + 1)], f32r, tag="lab")
            nc.sync.dma_start(out=lab_sb, in_=lab_d[:])

            bias_sb = small.tile([128, 1], f32, tag="bias")
            nc.sync.dma_start(out=bias_sb, in_=shv_d[:])

            ref_sb = {}
            for r in range(NREF):
                for c4 in range(NCHUNK):
                    for fh in range(2):
                        rt = refs.tile([128, CW], f32r, tag=f"ref{r}_{fh}_{c4}")
                        nc.sync.dma_start(out=rt, in_=ref_d[r, fh, c4])
                        ref_sb[(r, fh, c4)] = rt

            pred = predpool.tile([D + 1, T_LOC], f32)

            def label_mm(k, p):
                for cc in range(2):
                    nc.tensor.matmul(
                        pred[:, cc * 512:(cc + 1) * 512],
                        lhsT=lab_sb[:, k * (D + 1):(k + 1) * (D + 1)],
                        rhs=p[:, cc * 512:(cc + 1) * 512],
                        start=(k == 0), stop=(k == NT - 1),
                    )

            prev = None  # (k, p) pending label matmul, lagged one iteration
            for k in range(NT):
                r, hw = divmod(k, T // 128)   # hw in 0..31
                c4, h8 = divmod(hw, T // 128 // NCHUNK)
                sim = simpool.tile([128, T_LOC], f32, tag="sim")
                for fh in range(2):
                    lhsT = ref_sb[(r, fh, c4)][:, h8 * 128:(h8 + 1) * 128]
                    for cc in range(2):
                        nc.tensor.matmul(
                            sim[:, cc * 512:(cc + 1) * 512],
                            lhsT=lhsT,
                            rhs=tgt_sb[fh][:, cc * 512:(cc + 1) * 512],
                            start=(fh == 0), stop=(fh == 1),
                        )
                p = ppool.tile([128, T_LOC], f32r, tag="p")
                nc.scalar.activation(out=p, in_=sim,
                                     func=mybir.ActivationFunctionType.Exp,
                                     bias=bias_sb[:], scale=1.0)
                if prev is not None:
                    label_mm(*prev)
                prev = (k, p)
            label_mm(*prev)

            po = small.tile([D + 1, T_LOC], f32, tag="po")
            nc.vector.tensor_copy(po, pred)
            nc.sync.dma_start(out=out_d[:], in_=po)

    nc.compile()
    _CACHE["nc"] = nc
    return nc


def _prep_inputs(ref, target, ref_label):
    """Per-batch host-side relayouts shared by the 4 cores of each batch."""
    per_b = []
    for b in range(B):
        # [3, 2, 128, 4096] -> chunked [3, 2, 4, 128, 1024], contiguous
        refb = np.ascontiguousarray(
            ref[b].reshape(NREF, 2, 128, NCHUNK, CW).transpose(0, 1, 3, 2, 4)
        ).astype(np.float32, copy=False)
        # labels: n = (r, h, w) major -> [12288, 16], append ones -> [12288, 17]
        labn = ref_label[b].transpose(0, 2, 3, 1).reshape(N, D)
        labo = np.concatenate(
            [labn, np.ones((N, 1), np.float32)], axis=1).astype(np.float32)
        # -> SBUF layout [128, 96*17]: sb[p, k*17+j] = labo[k*128+p, j]
        labsb = np.ascontiguousarray(
            labo.reshape(NT, 128, D + 1).transpose(1, 0, 2).reshape(128, -1))
        tgtb = target[b].reshape(2, 128, T)
        per_b.append((refb, labsb, tgtb))
    return per_b


def _run_cores(per_b, shift):
    """One SPMD run with the given softmax shift; returns per-batch [17, 4096]."""
    global LAST_RESULTS
    from concourse.bass_utils import run_bass_kernel_spmd

    nc = _build_program()
    shv = np.full((128, 1), -shift, np.float32)
    in_maps = []
    for core in range(NCORES):
        b, s = divmod(core, NCORES // B)
        refb, labsb, tgtb = per_b[b]
        in_maps.append({
            "ref": refb,
            "tgt": np.ascontiguousarray(tgtb[:, :, s * T_LOC:(s + 1) * T_LOC]),
            "lab": labsb,
            "shv": shv,
        })
    LAST_RESULTS = run_bass_kernel_spmd(nc, in_maps, list(range(NCORES)))
    outs = LAST_RESULTS.results
    return [
        np.concatenate(
            [outs[b * (NCORES // B) + s]["out"] for s in range(NCORES // B)],
            axis=1).astype(np.float64)
        for b in range(B)
    ]


def _bad_cols(raw):
    """Columns whose exp window overflowed/underflowed for the used shift."""
    den, num = raw[D], raw[:D]
    return ~np.isfinite(den) | (den <= 0.0) | ~np.isfinite(num).all(axis=0)


def kernel(ref, target, ref_label, target_label):
    ref = np.asarray(ref, np.float32)
    target = np.asarray(target, np.float32)
    ref_label = np.asarray(ref_label, np.float32)
    labels = np.asarray(target_label).astype(np.int64)

    per_b = _prep_inputs(ref, target, ref_label)
    raws = _run_cores(per_b, SHIFT0)

    # Rescue any columns outside the exp window with shifted reruns (a no-op
    # for data resembling the reference distribution).
    bad = [_bad_cols(r) for r in raws]
    for delta in (60.0, -60.0, 120.0, -120.0):
        if not any(bm.any() for bm in bad):
            break
        raws2 = _run_cores(per_b, SHIFT0 + delta)
        for b in range(B):
            fixable = bad[b] & ~_bad_cols(raws2[b])
            raws[b][:, fixable] = raws2[b][:, fixable]
            bad[b] &= ~fixable

    nll_sum = 0.0
    for b in range(B):
        pred = raws[b][:D] / raws[b][D]                      # [16, 4096]
        logp = np.log(pred + EPS)
        idx = labels[b].reshape(T)
        nll_sum += -logp[idx, np.arange(T)].sum()
    loss = nll_sum / (B * T)
    return np.asarray(loss, dtype=np.float32)


# revision 13
# speedup vs baseline: 1131.5734x; 1131.5734x over previous
"""Fused cross-entropy label-propagation kernel for Trainium2 (8 cores).

Computation (per batch b):
  sim   = ref_flat(b) @ tgt_flat(b)          # [12288, 4096]
  prob  = softmax(sim, axis=0)               # over ref pixels
  pred  = lab_flat(b) @ prob                 # [16, 4096]
  loss  = mean(-log(pred[label] + eps))

Sharding: batch b = core // 4, target-pixel columns split 4-way per batch
(softmax is over the ref axis, so column sharding needs no communication).

Device kernel (per core, T_LOC = 1024 columns):
  for each of 96 ref-row tiles (128 rows):
    sim_psum[128, 1024]  = refT_tile.T @ tgt          (2 accumulating matmuls, K=256)
    p[128, 1024]         = exp(sim_psum - 80)         (ScalarE, reads PSUM)
    pred_psum[17, 1024] += [lab|ones]_tile.T @ p      (accumulate over all 96 tiles)
  out = pred_psum   # rows 0..15 = unnormalized label sums, row 16 = softmax denom

The constant shift replaces the per-column max. The data contains near-collinear
ref/target pairs, so per-column sim maxima span roughly [57, 221] — that 164-wide
log-range just fits inside float32's ~175-unit representable exp window when
centered with shift ~138.5. The shift is a kernel INPUT: the host validates every
column of the result (denominator finite and positive) and, if any column falls
outside the window, reruns with the shift moved +-60 and patches those columns.
The shift cancels exactly in num/den.

Host finishes with num/den, log, gather, mean over 8192 elements (float64).
"""

import numpy as np

B, NREF, F, H, W, D = 2, 3, 256, 64, 64, 16
T = H * W                     # 4096 target pixels per batch
N = NREF * T                  # 12288 ref pixels per batch
NCORES = 8
T_LOC = B * T // NCORES       # 1024 columns per core
NT = N // 128                 # 96 ref-row tiles
NCHUNK = 4                    # ref DMA chunks per (r, fh) tile
CW = T // NCHUNK              # 1024 columns per ref chunk
SHIFT0 = 138.5                # subtracted from sim before exp (host-adjustable)
EPS = 1e-14

_CACHE = {}
LAST_RESULTS = None  # BassKernelResults of the most recent run (for profiling)


def _build_program(reps=1):
    # reps > 1 repeats the whole compute body (timing harness only; the extra
    # reps recompute the same result into the same output).
    if ("nc", reps) in _CACHE:
        return _CACHE[("nc", reps)]

    import concourse.bacc as bacc
    import concourse.tile as tile
    import concourse.mybir as mybir

    f32 = mybir.dt.float32
    f32r = mybir.dt.float32r

    nc = bacc.Bacc("TRN2", target_bir_lowering=False, debug=False,
                   num_devices=NCORES)

    # Per-core inputs, pre-laid-out on host so every DMA is contiguous.
    ref_d = nc.dram_tensor("ref", [NREF, 2, NCHUNK, 128, CW], f32r,
                           kind="ExternalInput")
    tgt_d = nc.dram_tensor("tgt", [2, 128, T_LOC], f32r, kind="ExternalInput")
    lab_d = nc.dram_tensor("lab", [128, NT * (D + 1)], f32r,
                           kind="ExternalInput")
    shv_d = nc.dram_tensor("shv", [128, 1], f32, kind="ExternalInput")
    out_d = nc.dram_tensor("out", [D + 1, T_LOC], f32, kind="ExternalOutput")

    with tile.TileContext(nc) as tc:
        with (
            tc.tile_pool(name="refs", bufs=1) as refs,
            tc.tile_pool(name="small", bufs=1) as small,
            tc.tile_pool(name="ppool", bufs=3) as ppool,
            tc.tile_pool(name="simpool", bufs=3, space="PSUM") as simpool,
            tc.tile_pool(name="predpool", bufs=1, space="PSUM") as predpool,
        ):
            # Loads needed by the first iteration go first.
            tgt_sb = []
            for fh in range(2):
                t_ = small.tile([128, T_LOC], f32r, tag=f"tgt{fh}")
                nc.sync.dma_start(out=t_, in_=tgt_d[fh])
                tgt_sb.append(t_)
            lab_sb = small.tile([128, NT * (D + 1)], f32r, tag="lab")
            nc.sync.dma_start(out=lab_sb, in_=lab_d[:])

            bias_sb = small.tile([128, 1], f32, tag="bias")
            nc.sync.dma_start(out=bias_sb, in_=shv_d[:])

            ref_sb = {}
            for r in range(NREF):
                for c4 in range(NCHUNK):
                    for fh in range(2):
                        rt = refs.tile([128, CW], f32r, tag=f"ref{r}_{fh}_{c4}")
                        nc.sync.dma_start(out=rt, in_=ref_d[r, fh, c4])
                        ref_sb[(r, fh, c4)] = rt

            for rep in range(reps):
                pred = predpool.tile([D + 1, T_LOC], f32, tag="pred")

                def label_mm(k, p, pred=pred):
                    for cc in range(2):
                        nc.tensor.matmul(
                            pred[:, cc * 512:(cc + 1) * 512],
                            lhsT=lab_sb[:, k * (D + 1):(k + 1) * (D + 1)],
                            rhs=p[:, cc * 512:(cc + 1) * 512],
                            start=(k == 0), stop=(k == NT - 1),
                        )

                prev = None  # (k, p) pending label matmul, lagged one iteration
                for k in range(NT):
                    r, hw = divmod(k, T // 128)   # hw in 0..31
                    c4, h8 = divmod(hw, T // 128 // NCHUNK)
                    sim = simpool.tile([128, T_LOC], f32, tag="sim")
                    for fh in range(2):
                        lhsT = ref_sb[(r, fh, c4)][:, h8 * 128:(h8 + 1) * 128]
                        for cc in range(2):
                            nc.tensor.matmul(
                                sim[:, cc * 512:(cc + 1) * 512],
                                lhsT=lhsT,
                                rhs=tgt_sb[fh][:, cc * 512:(cc + 1) * 512],
                                start=(fh == 0), stop=(fh == 1),
                            )
                    p = ppool.tile([128, T_LOC], f32r, tag="p")
                    nc.scalar.activation(out=p, in_=sim,
                                         func=mybir.ActivationFunctionType.Exp,
                                         bias=bias_sb[:], scale=1.0)
                    if prev is not None:
                        label_mm(*prev)
                    prev = (k, p)
                label_mm(*prev)

                po = small.tile([D + 1, T_LOC], f32, tag="po")
                nc.vector.tensor_copy(po, pred)
                nc.sync.dma_start(out=out_d[:], in_=po)

    nc.compile()
    _CACHE[("nc", reps)] = nc
    return nc


def _prep_inputs(ref, target, ref_label):
    """Per-batch host-side relayouts shared by the 4 cores of each batch."""
    per_b = []
    for b in range(B):
        # [3, 2, 128, 4096] -> chunked [3, 2, 4, 128, 1024], contiguous
        refb = np.ascontiguousarray(
            ref[b].reshape(NREF, 2, 128, NCHUNK, CW).transpose(0, 1, 3, 2, 4)
        ).astype(np.float32, copy=False)
        # labels: n = (r, h, w) major -> [12288, 16], append ones -> [12288, 17]
        labn = ref_label[b].transpose(0, 2, 3, 1).reshape(N, D)
        labo = np.concatenate(
            [labn, np.ones((N, 1), np.float32)], axis=1).astype(np.float32)
        # -> SBUF layout [128, 96*17]: sb[p, k*17+j] = labo[k*128+p, j]
        labsb = np.ascontiguousarray(
            labo.reshape(NT, 128, D + 1).transpose(1, 0, 2).reshape(128, -1))
        tgtb = target[b].reshape(2, 128, T)
        per_b.append((refb, labsb, tgtb))
    return per_b


def _run_cores(per_b, shift):
    """One SPMD run with the given softmax shift; returns per-batch [17, 4096]."""
    global LAST_RESULTS
    from concourse.bass_utils import run_bass_kernel_spmd

    nc = _build_program()
    shv = np.full((128, 1), -shift, np.float32)
    in_maps = []
    for core in range(NCORES):
        b, s = divmod(core, NCORES // B)
        refb, labsb, tgtb = per_b[b]
        in_maps.append({
            "ref": refb,
            "tgt": np.ascontiguousarray(tgtb[:, :, s * T_LOC:(s + 1) * T_LOC]),
            "lab": labsb,
            "shv": shv,
        })
    LAST_RESULTS = run_bass_kernel_spmd(nc, in_maps, list(range(NCORES)))
    outs = LAST_RESULTS.results
    return [
        np.concatenate(
            [outs[b * (NCORES // B) + s]["out"] for s in range(NCORES // B)],
            axis=1).astype(np.float64)
        for b in range(B)
    ]


def _bad_cols(raw):
    """Columns whose exp window overflowed/underflowed for the used shift."""
    den, num = raw[D], raw[:D]
    return ~np.isfinite(den) | (den <= 0.0) | ~np.isfinite(num).all(axis=0)


def kernel(ref, target, ref_label, target_label):
    ref = np.asarray(ref, np.float32)
    target = np.asarray(target, np.float32)
    ref_label = np.asarray(ref_label, np.float32)
    labels = np.asarray(target_label).astype(np.int64)

    per_b = _prep_inputs(ref, target, ref_label)
    raws = _run_cores(per_b, SHIFT0)

    # Rescue any columns outside the exp window with shifted reruns (a no-op
    # for data resembling the reference distribution).
    bad = [_bad_cols(r) for r in raws]
    for delta in (60.0, -60.0, 120.0, -120.0):
        if not any(bm.any() for bm in bad):
            break
        raws2 = _run_cores(per_b, SHIFT0 + delta)
        for b in range(B):
            fixable = bad[b] & ~_bad_cols(raws2[b])
            raws[b][:, fixable] = raws2[b][:, fixable]
            bad[b] &= ~fixable

    nll_sum = 0.0
    for b in range(B):
        pred = raws[b][:D] / raws[b][D]                      # [16, 4096]
        logp = np.log(pred + EPS)
        idx = labels[b].reshape(T)
        nll_sum += -logp[idx, np.arange(T)].sum()
    loss = nll_sum / (B * T)
    return np.asarray(loss, dtype=np.float32)


# revision 16
# speedup vs baseline: 1140.8807x; 1.0082x over previous
"""Fused cross-entropy label-propagation kernel for Trainium2 (8 cores).

Computation (per batch b):
  sim   = ref_flat(b) @ tgt_flat(b)          # [12288, 4096]
  prob  = softmax(sim, axis=0)               # over ref pixels
  pred  = lab_flat(b) @ prob                 # [16, 4096]
  loss  = mean(-log(pred[label] + eps))

Sharding: batch b = core // 4, target-pixel columns split 4-way per batch
(softmax is over the ref axis, so column sharding needs no communication).

Device kernel (per core, T_LOC = 1024 columns):
  for each of 96 ref-row tiles (128 rows):
    sim_psum[128, 1024]  = refT_tile.T @ tgt          (2 accumulating matmuls, K=256)
    p[128, 1024]         = exp(sim_psum - 80)         (ScalarE, reads PSUM)
    pred_psum[17, 1024] += [lab|ones]_tile.T @ p      (accumulate over all 96 tiles)
  out = pred_psum   # rows 0..15 = unnormalized label sums, row 16 = softmax denom

The constant shift replaces the per-column max. The data contains near-collinear
ref/target pairs, so per-column sim maxima span roughly [57, 221] — that 164-wide
log-range just fits inside float32's ~175-unit representable exp window when
centered with shift ~138.5. The shift is a kernel INPUT: the host validates every
column of the result (denominator finite and positive) and, if any column falls
outside the window, reruns with the shift moved +-60 and patches those columns.
The shift cancels exactly in num/den.

Host finishes with num/den, log, gather, mean over 8192 elements (float64).
"""

import numpy as np

B, NREF, F, H, W, D = 2, 3, 256, 64, 64, 16
T = H * W                     # 4096 target pixels per batch
N = NREF * T                  # 12288 ref pixels per batch
NCORES = 8
T_LOC = B * T // NCORES       # 1024 columns per core
NT = N // 128                 # 96 ref-row tiles
NCHUNK = 4                    # ref DMA chunks per (r, fh) tile
CW = T // NCHUNK              # 1024 columns per ref chunk
SHIFT0 = 138.5                # subtracted from sim before exp (host-adjustable)
EPS = 1e-14

_CACHE = {}
LAST_RESULTS = None  # BassKernelResults of the most recent run (for profiling)


def _build_program(reps=1):
    # reps > 1 repeats the whole compute body (timing harness only; the extra
    # reps recompute the same result into the same output).
    if ("nc", reps) in _CACHE:
        return _CACHE[("nc", reps)]

    import concourse.bacc as bacc
    import concourse.tile as tile
    import concourse.mybir as mybir

    f32 = mybir.dt.float32
    f32r = mybir.dt.float32r

    nc = bacc.Bacc("TRN2", target_bir_lowering=False, debug=False,
                   num_devices=NCORES)

    # Per-core inputs, pre-laid-out on host so every DMA is contiguous.
    ref_d = nc.dram_tensor("ref", [NREF, 2, NCHUNK, 128, CW], f32r,
                           kind="ExternalInput")
    tgt_d = nc.dram_tensor("tgt", [2, 128, T_LOC], f32r, kind="ExternalInput")
    lab_d = nc.dram_tensor("lab", [128, NT * (D + 1)], f32r,
                           kind="ExternalInput")
    shv_d = nc.dram_tensor("shv", [128, 1], f32, kind="ExternalInput")
    out_d = nc.dram_tensor("out", [D + 1, T_LOC], f32, kind="ExternalOutput")

    with tile.TileContext(nc) as tc:
        with (
            tc.tile_pool(name="refs", bufs=1) as refs,
            tc.tile_pool(name="small", bufs=1) as small,
            tc.tile_pool(name="ppool", bufs=3) as ppool,
            tc.tile_pool(name="simpool", bufs=3, space="PSUM") as simpool,
            tc.tile_pool(name="predpool", bufs=1, space="PSUM") as predpool,
        ):
            # Loads needed by the first iteration go first.
            tgt_sb = []
            for fh in range(2):
                t_ = small.tile([128, T_LOC], f32r, tag=f"tgt{fh}")
                nc.sync.dma_start(out=t_, in_=tgt_d[fh])
                tgt_sb.append(t_)
            bias_sb = small.tile([128, 1], f32, tag="bias")
            nc.sync.dma_start(out=bias_sb, in_=shv_d[:])

            # ref chunks in consumption order; labels aren't needed until the
            # first label matmul, so their DMA goes after the first chunk pair.
            ref_sb = {}
            lab_sb = None
            for r in range(NREF):
                for c4 in range(NCHUNK):
                    for fh in range(2):
                        rt = refs.tile([128, CW], f32r, tag=f"ref{r}_{fh}_{c4}")
                        nc.sync.dma_start(out=rt, in_=ref_d[r, fh, c4])
                        ref_sb[(r, fh, c4)] = rt
                    if lab_sb is None:
                        lab_sb = small.tile([128, NT * (D + 1)], f32r, tag="lab")
                        nc.sync.dma_start(out=lab_sb, in_=lab_d[:])

            for rep in range(reps):
                pred = predpool.tile([D + 1, T_LOC], f32, tag="pred")

                def label_mm(k, p, pred=pred):
                    for cc in range(2):
                        nc.tensor.matmul(
                            pred[:, cc * 512:(cc + 1) * 512],
                            lhsT=lab_sb[:, k * (D + 1):(k + 1) * (D + 1)],
                            rhs=p[:, cc * 512:(cc + 1) * 512],
                            start=(k == 0), stop=(k == NT - 1),
                        )

                prev = None  # (k, p) pending label matmul, lagged one iteration
                for k in range(NT):
                    r, hw = divmod(k, T // 128)   # hw in 0..31
                    c4, h8 = divmod(hw, T // 128 // NCHUNK)
                    sim = simpool.tile([128, T_LOC], f32, tag="sim")
                    for fh in range(2):
                        lhsT = ref_sb[(r, fh, c4)][:, h8 * 128:(h8 + 1) * 128]
                        for cc in range(2):
                            nc.tensor.matmul(
                                sim[:, cc * 512:(cc + 1) * 512],
                                lhsT=lhsT,
                                rhs=tgt_sb[fh][:, cc * 512:(cc + 1) * 512],
                                start=(fh == 0), stop=(fh == 1),
                            )
                    p = ppool.tile([128, T_LOC], f32r, tag="p")
                    nc.scalar.activation(out=p, in_=sim,
                                         func=mybir.ActivationFunctionType.Exp,
                                         bias=bias_sb[:], scale=1.0)
                    if prev is not None:
                        label_mm(*prev)
                    prev = (k, p)
                label_mm(*prev)

                po = small.tile([D + 1, T_LOC], f32, tag="po")
                nc.vector.tensor_copy(po, pred)
                nc.sync.dma_start(out=out_d[:], in_=po)

    nc.compile()
    _CACHE[("nc", reps)] = nc
    return nc


def _prep_inputs(ref, target, ref_label):
    """Per-batch host-side relayouts shared by the 4 cores of each batch."""
    per_b = []
    for b in range(B):
        # [3, 2, 128, 4096] -> chunked [3, 2, 4, 128, 1024], contiguous
        refb = np.ascontiguousarray(
            ref[b].reshape(NREF, 2, 128, NCHUNK, CW).transpose(0, 1, 3, 2, 4)
        ).astype(np.float32, copy=False)
        # labels: n = (r, h, w) major -> [12288, 16], append ones -> [12288, 17]
        labn = ref_label[b].transpose(0, 2, 3, 1).reshape(N, D)
        labo = np.concatenate(
            [labn, np.ones((N, 1), np.float32)], axis=1).astype(np.float32)
        # -> SBUF layout [128, 96*17]: sb[p, k*17+j] = labo[k*128+p, j]
        labsb = np.ascontiguousarray(
            labo.reshape(NT, 128, D + 1).transpose(1, 0, 2).reshape(128, -1))
        tgtb = target[b].reshape(2, 128, T)
        per_b.append((refb, labsb, tgtb))
    return per_b


def _run_cores(per_b, shift):
    """One SPMD run with the given softmax shift; returns per-batch [17, 4096]."""
    global LAST_RESULTS
    from concourse.bass_utils import run_bass_kernel_spmd

    nc = _build_program()
    shv = np.full((128, 1), -shift, np.float32)
    in_maps = []
    for core in range(NCORES):
        b, s = divmod(core, NCORES // B)
        refb, labsb, tgtb = per_b[b]
        in_maps.append({
            "ref": refb,
            "tgt": np.ascontiguousarray(tgtb[:, :, s * T_LOC:(s + 1) * T_LOC]),
            "lab": labsb,
            "shv": shv,
        })
    LAST_RESULTS = run_bass_kernel_spmd(nc, in_maps, list(range(NCORES)))
    outs = LAST_RESULTS.results
    return [
        np.concatenate(
            [outs[b * (NCORES // B) + s]["out"] for s in range(NCORES // B)],
            axis=1).astype(np.float64)
        for b in range(B)
    ]


def _bad_cols(raw):
    """Columns whose exp window overflowed/underflowed for the used shift."""
    with np.errstate(all="ignore"):
        den, num = raw[D], raw[:D]
        return ~np.isfinite(den) | (den <= 0.0) | ~np.isfinite(num).all(axis=0)


def kernel(ref, target, ref_label, target_label):
    ref = np.asarray(ref, np.float32)
    target = np.asarray(target, np.float32)
    ref_label = np.asarray(ref_label, np.float32)
    labels = np.asarray(target_label).astype(np.int64)

    per_b = _prep_inputs(ref, target, ref_label)
    raws = _run_cores(per_b, SHIFT0)

    # Rescue any columns outside the exp window with shifted reruns (a no-op
    # for data resembling the reference distribution).
    bad = [_bad_cols(r) for r in raws]
    for delta in (60.0, -60.0, 120.0, -120.0):
        if not any(bm.any() for bm in bad):
            break
        raws2 = _run_cores(per_b, SHIFT0 + delta)
        for b in range(B):
            fixable = bad[b] & ~_bad_cols(raws2[b])
            raws[b][:, fixable] = raws2[b][:, fixable]
            bad[b] &= ~fixable

    nll_sum = 0.0
    with np.errstate(all="ignore"):
        for b in range(B):
            pred = raws[b][:D] / raws[b][D]                  # [16, 4096]
            logp = np.log(pred + EPS)
            idx = labels[b].reshape(T)
            nll_sum += -logp[idx, np.arange(T)].sum()
    loss = nll_sum / (B * T)
    return np.asarray(loss, dtype=np.float32)
